# revision 1
# baseline (speedup 1.0000x reference)
"""Self-contained Trainium2 kernel for nn_ContextualizedNN (gnn_message_passing).

kernel(**inputs) takes the FULL unsharded inputs (as produced by the problem's
setup_inputs) and returns the full [8192] float32 output.

Strategy: data-parallel over the batch across 8 NeuronCores. Each core gets the
full tables (replicated) and a 1024-element slice of the batch. All model
compute (neighbor-index gather, score/embedding gathers, scored = S @ E,
MLP head) runs on-device in Bass; the host only shards inputs, packs the MLP
weights into the kernel's SBUF layout (bf16), and concatenates core outputs.

Per-core pipeline (per tile of TB=128 batch elements):
  stage 0: gather idx_tensor rows for the tile's batch indices
           -> [128(e), K] i32 -> cast f32 -> PE transpose -> idxT [K, 128(e)] i32
  stage 1: per 32-element subtile, one indirect DMA gathers all score rows
           (S_all [K(k)p, 32*K] f32) and one gathers all embedding rows
           (E_all [K(j)p, 32*D] f32); E is cast to bf16 (ACT).
  stage 2: per element, PE-transpose S_e (fp32, batched 5 per PSUM bank),
           DVE copy-cast -> bf16 S_T; mm1 scored[k,d] = S_T.T @ E_bf (bf16,
           fp32 PSUM, batched 8 per bank); ACT copy-cast -> T'[K(k), (d, e)] bf16.
  stage 3: mm2: H[h,e] += W1p[k, (side,d,h)].T @ T'[k, d-slice] over side,d
           (128 accumulating matmuls); relu(+b1) -> bf16; o = W2.T @ r;
           relu(+b2); sigmoid -> out.
"""
import os
os.environ.setdefault("JAX_PLATFORMS", "cpu")
from contextlib import ExitStack

import numpy as np
import ml_dtypes

import concourse.bass as bass
import concourse.bacc as bacc
import concourse.tile as tile
from concourse import mybir
from concourse.bass_utils import run_bass_kernel_spmd
from concourse.masks import make_identity

P = 128
K = 100
D = 64
HID = 128
N_USERS = 100000
N_ITEMS = 50000
B = 8192
N_CORES = 8
B_CORE = B // N_CORES
TB = 128     # batch tile (mm2 group)
USE_CONCAT = True  # False: gather scr/emb separately from fp32 tables (slower, proven)
SUB = 32     # elements per payload gather

F32 = mybir.dt.float32
BF16 = mybir.dt.bfloat16
I32 = mybir.dt.int32


def _build(n_users, n_items, b_core, tb, sub, use_concat=True):
    nc = bacc.Bacc("TRN2", target_bir_lowering=False, debug=False)

    u_idx = nc.dram_tensor("u_idx", [b_core, 1], I32, kind="ExternalInput").ap()
    i_idx = nc.dram_tensor("i_idx", [b_core, 1], I32, kind="ExternalInput").ap()
    u_idxt = nc.dram_tensor("u_idxt", [n_users, K], I32, kind="ExternalInput").ap()
    i_idxt = nc.dram_tensor("i_idxt", [n_items, K], I32, kind="ExternalInput").ap()
    u_scr = nc.dram_tensor("u_scr", [n_users, K], F32, kind="ExternalInput").ap()
    i_scr = nc.dram_tensor("i_scr", [n_items, K], F32, kind="ExternalInput").ap()
    u_emb = nc.dram_tensor("u_emb", [n_users, D], F32, kind="ExternalInput").ap()
    i_emb = nc.dram_tensor("i_emb", [n_items, D], F32, kind="ExternalInput").ap()
    w1p = nc.dram_tensor("w1p", [K, 2 * D * HID], BF16, kind="ExternalInput").ap()
    b1p = nc.dram_tensor("b1p", [HID, 1], F32, kind="ExternalInput").ap()
    w2p = nc.dram_tensor("w2p", [HID, 1], BF16, kind="ExternalInput").ap()
    b2p = nc.dram_tensor("b2p", [1, 1], F32, kind="ExternalInput").ap()
    out = nc.dram_tensor("out", [1, b_core], F32, kind="ExternalOutput").ap()
    CW = K + D  # concat row: [scr | emb] in bf16
    tcat_u = nc.dram_tensor("tcat_u", [n_users, CW], BF16).ap()
    tcat_i = nc.dram_tensor("tcat_i", [n_items, CW], BF16).ap()

    sides = [
        dict(idx=u_idx, idxt=u_idxt, scr=u_scr, emb=u_emb, cat=tcat_u,
             name="u", si=0),
        dict(idx=i_idx, idxt=i_idxt, scr=i_scr, emb=i_emb, cat=tcat_i,
             name="v", si=1),
    ]
    n_tiles = b_core // tb
    n_sub = tb // sub

    with tile.TileContext(nc) as tc:
        ctx = ExitStack()
        consts = ctx.enter_context(tc.tile_pool(name="consts", bufs=1))
        idxp = ctx.enter_context(tc.tile_pool(name="idxp", bufs=2))
        gath = ctx.enter_context(tc.tile_pool(name="gath", bufs=6 if use_concat else 2))
        stp = ctx.enter_context(tc.tile_pool(name="stp", bufs=4))
        tpp = ctx.enter_context(tc.tile_pool(name="tpp", bufs=2))
        outp = ctx.enter_context(tc.tile_pool(name="outp", bufs=2))
        psp = ctx.enter_context(tc.tile_pool(name="psp", bufs=2, space="PSUM"))
        psh = ctx.enter_context(tc.tile_pool(name="psh", bufs=2, space="PSUM"))

        w1sb = consts.tile([P, 2 * D * HID], BF16)
        nc.sync.dma_start(out=w1sb[:K, :], in_=w1p[:, :])
        b1sb = consts.tile([P, 1], F32)
        nc.sync.dma_start(out=b1sb[:HID, :], in_=b1p[:, :])
        w2sb = consts.tile([P, 1], BF16)
        nc.sync.dma_start(out=w2sb[:HID, :], in_=w2p[:, :])
        b2sb = consts.tile([P, 1], F32)
        nc.sync.dma_start(out=b2sb[:1, :], in_=b2p[:, :])
        ident = consts.tile([P, P], F32)
        make_identity(nc, ident[:])
        identb = consts.tile([P, P], BF16)
        make_identity(nc, identb[:])

        # build interleaved bf16 [scr | emb] tables in DRAM (SWDGE cast).
        # split into <16384-descriptor chunks (one descriptor per row).
        BCH = 8000
        for sd in (sides if use_concat else []):
            nrows = sd["cat"].shape[0]
            for r0 in range(0, nrows, BCH):
                r1 = min(r0 + BCH, nrows)
                nc.gpsimd.dma_start(
                    out=sd["cat"][r0:r1, 0:K], in_=sd["scr"][r0:r1, :]
                )
                nc.gpsimd.dma_start(
                    out=sd["cat"][r0:r1, K:K + D], in_=sd["emb"][r0:r1, :]
                )

        for t in range(n_tiles):
            tprimes = []
            for sd in sides:
                bidx = idxp.tile([P, 1], I32, tag="bidx")
                nc.sync.dma_start(
                    out=bidx[:tb, :], in_=sd["idx"][t * tb:(t + 1) * tb, :]
                )
                irows = idxp.tile([P, K], I32, tag="irows")
                nc.gpsimd.indirect_dma_start(
                    out=irows[:tb, :],
                    out_offset=None,
                    in_=sd["idxt"][:, :],
                    in_offset=bass.IndirectOffsetOnAxis(ap=bidx[:tb, :1], axis=0),
                )
                irows_f = idxp.tile([P, K], F32, tag="irows_f")
                nc.vector.tensor_copy(out=irows_f[:tb, :], in_=irows[:tb, :])
                it_ps = psp.tile([P, tb], F32, space="PSUM", tag="it_ps")
                nc.tensor.transpose(
                    out=it_ps[:K, :tb], in_=irows_f[:tb, :K],
                    identity=ident[:tb, :tb],
                )
                idxT = idxp.tile([P, tb], I32, tag="idxT")
                nc.vector.tensor_copy(out=idxT[:K, :tb], in_=it_ps[:K, :tb])

                tprime = tpp.tile([P, D * tb], BF16, tag=f"tp{sd['name']}")
                tprimes.append(tprime)
                for s in range(n_sub):
                    e0 = s * sub
                    if use_concat:
                        grp = gath.tile([P, sub * (K + D)], BF16, tag="grp")
                        for i in range(sub):
                            nc.gpsimd.indirect_dma_start(
                                out=grp[:K, i * (K + D):(i + 1) * (K + D)],
                                out_offset=None,
                                in_=sd["cat"][:, :],
                                in_offset=bass.IndirectOffsetOnAxis(
                                    ap=idxT[:K, e0 + i:e0 + i + 1], axis=0
                                ),
                            )
                    else:
                        sgrp = gath.tile([P, sub * K], F32, tag="sgrp")
                        egrp = gath.tile([P, sub * D], F32, tag="egrp")
                        for i in range(sub):
                            nc.gpsimd.indirect_dma_start(
                                out=sgrp[:K, i * K:(i + 1) * K],
                                out_offset=None,
                                in_=sd["scr"][:, :],
                                in_offset=bass.IndirectOffsetOnAxis(
                                    ap=idxT[:K, e0 + i:e0 + i + 1], axis=0
                                ),
                            )
                            nc.gpsimd.indirect_dma_start(
                                out=egrp[:K, i * D:(i + 1) * D],
                                out_offset=None,
                                in_=sd["emb"][:, :],
                                in_offset=bass.IndirectOffsetOnAxis(
                                    ap=idxT[:K, e0 + i:e0 + i + 1], axis=0
                                ),
                            )
                        e_bf = gath.tile([P, sub * D], BF16, tag="e_bf")
                        nc.scalar.copy(out=e_bf[:K, :], in_=egrp[:K, :])

                    for q0 in range(0, sub, 5):
                        qn = min(5, sub - q0)
                        st_ps = psp.tile(
                            [P, 5 * K], BF16 if use_concat else F32,
                            space="PSUM", tag="st_ps",
                        )
                        for q in range(qn):
                            e = q0 + q
                            if use_concat:
                                tin = grp[:K, e * (K + D):e * (K + D) + K]
                                tid = identb[:K, :K]
                            else:
                                tin = sgrp[:K, e * K:(e + 1) * K]
                                tid = ident[:K, :K]
                            nc.tensor.transpose(
                                out=st_ps[:K, q * K:(q + 1) * K],
                                in_=tin, identity=tid,
                            )
                        st_sb = stp.tile([P, 5 * K], BF16, tag="st_sb")
                        nc.vector.tensor_copy(
                            out=st_sb[:K, :qn * K], in_=st_ps[:K, :qn * K]
                        )
                        for q in range(qn):
                            e = q0 + q
                            r = e % 8
                            if r == 0:
                                sc_ps = psp.tile(
                                    [P, 8 * D], F32, space="PSUM", tag="sc_ps"
                                )
                            nc.tensor.matmul(
                                out=sc_ps[:K, r * D:(r + 1) * D],
                                lhsT=st_sb[:K, q * K:(q + 1) * K],
                                rhs=(
                                    grp[:K, e * (K + D) + K:(e + 1) * (K + D)]
                                    if use_concat else
                                    e_bf[:K, e * D:(e + 1) * D]
                                ),
                                start=True, stop=True,
                            )
                            if r == 7 or e == sub - 1:
                                rn = r + 1
                                eb = e0 + e - r
                                tp_v = (
                                    tprime[:K, :]
                                    .rearrange("p (d e) -> p d e", d=D)
                                    .transpose([0, 2, 1])[:, eb:eb + rn, :]
                                )
                                sc_v = sc_ps[:K, :rn * D].rearrange(
                                    "p (e d) -> p e d", d=D
                                )
                                nc.scalar.copy(out=tp_v, in_=sc_v)

            h_ps = psh.tile([P, tb], F32, space="PSUM", tag="h_ps")
            nmm = 2 * D
            m = 0
            for si, tprime in enumerate(tprimes):
                for d in range(D):
                    nc.tensor.matmul(
                        out=h_ps[:HID, :tb],
                        lhsT=w1sb[
                            :K, si * D * HID + d * HID:si * D * HID + (d + 1) * HID
                        ],
                        rhs=tprime[:K, d * tb:(d + 1) * tb],
                        start=(m == 0), stop=(m == nmm - 1),
                    )
                    m += 1
            r_sb = outp.tile([P, tb], BF16, tag="r_sb")
            nc.scalar.activation(
                out=r_sb[:HID, :tb], in_=h_ps[:HID, :tb],
                func=mybir.ActivationFunctionType.Relu,
                bias=b1sb[:HID, :1], scale=1.0,
            )
            o_ps = psh.tile([P, tb], F32, space="PSUM", tag="h_ps")
            nc.tensor.matmul(
                out=o_ps[:1, :tb], lhsT=w2sb[:HID, :1], rhs=r_sb[:HID, :tb],
                start=True, stop=True,
            )
            o1 = outp.tile([P, tb], F32, tag="o1")
            nc.scalar.activation(
                out=o1[:1, :tb], in_=o_ps[:1, :tb],
                func=mybir.ActivationFunctionType.Relu,
                bias=b2sb[:1, :1], scale=1.0,
            )
            o2 = outp.tile([P, tb], F32, tag="o2")
            nc.scalar.activation(
                out=o2[:1, :tb], in_=o1[:1, :tb],
                func=mybir.ActivationFunctionType.Sigmoid,
            )
            nc.sync.dma_start(out=out[:1, t * tb:(t + 1) * tb], in_=o2[:1, :tb])
        ctx.close()

    nc.compile()
    return nc


_NC_CACHE = {}


def _get_nc():
    key = (N_USERS, N_ITEMS, B_CORE, TB, SUB, USE_CONCAT)
    if key not in _NC_CACHE:
        _NC_CACHE[key] = _build(N_USERS, N_ITEMS, B_CORE, TB, SUB, USE_CONCAT)
    return _NC_CACHE[key]


def _pack_weights(W1, b1, W2, b2):
    w1p = np.ascontiguousarray(
        np.asarray(W1, np.float32)
        .reshape(2, K, D, HID).transpose(1, 0, 2, 3).reshape(K, 2 * D * HID)
        .astype(ml_dtypes.bfloat16)
    )
    w2p = np.ascontiguousarray(
        np.asarray(W2, np.float32).reshape(HID, 1).astype(ml_dtypes.bfloat16)
    )
    b1p = np.ascontiguousarray(np.asarray(b1, np.float32).reshape(HID, 1))
    b2p = np.ascontiguousarray(np.asarray(b2, np.float32).reshape(1, 1))
    return w1p, b1p, w2p, b2p


def kernel(user_idxs, item_idxs, user_idx_tensor, item_idx_tensor,
           user_scr_tensor, item_scr_tensor, user_emb, item_emb,
           W1, b1, W2, b2, _trace=False):
    nc = _get_nc()
    w1p, b1p, w2p, b2p = _pack_weights(W1, b1, W2, b2)

    u_idx = np.ascontiguousarray(np.asarray(user_idxs).astype(np.int32)[:, None])
    i_idx = np.ascontiguousarray(np.asarray(item_idxs).astype(np.int32)[:, None])
    common = dict(
        u_idxt=np.ascontiguousarray(np.asarray(user_idx_tensor, np.int32)),
        i_idxt=np.ascontiguousarray(np.asarray(item_idx_tensor, np.int32)),
        u_scr=np.ascontiguousarray(np.asarray(user_scr_tensor, np.float32)),
        i_scr=np.ascontiguousarray(np.asarray(item_scr_tensor, np.float32)),
        u_emb=np.ascontiguousarray(np.asarray(user_emb, np.float32)),
        i_emb=np.ascontiguousarray(np.asarray(item_emb, np.float32)),
        w1p=w1p, b1p=b1p, w2p=w2p, b2p=b2p,
    )
    in_maps = []
    for c in range(N_CORES):
        m = dict(common)
        m["u_idx"] = u_idx[c * B_CORE:(c + 1) * B_CORE]
        m["i_idx"] = i_idx[c * B_CORE:(c + 1) * B_CORE]
        in_maps.append(m)

    res = run_bass_kernel_spmd(nc, in_maps, list(range(N_CORES)), trace=_trace)
    out = np.concatenate([res.results[c]["out"][0] for c in range(N_CORES)])
    if _trace:
        kernel._last_exec_time_ns = res.exec_time_ns
        kernel._last_results = res
    return out

